# revision 29
# baseline (speedup 1.0000x reference)
"""Trainium2 Bass kernel for nn_Decision_Node (Linear+Hardtanh -> sp, 2-class
softmax Gini -> gini), data-parallel over 8 NeuronCores.

Math per core shard (B_s=128 of B=1024 batches, T=128, F=784, L=256, C=2):
    sp   = clip(x @ W.T + b, -1, 1)                      [N=16384, 256]
    gini = 1.5 - 0.5*tanh(sp*d/2)^2,  d = contrib[...,0]-contrib[...,1]

Device strategy (v2, W-stationary):
  - x cast to fp16 on host, column-blocked+padded to [7, 128, N] with a
    bias-fold row (xt[6,16,:] = 1.0 pairs with wt[6,16,:] = b).
  - Matmuls keep W tiles stationary ([128f x 128l] from resident SBUF) and
    stream 512-row x chunks; out psum is [128l x 512r] (one full bank).
  - ACT reads each psum chunk once: sp_u8 = sat_u8(round(127.5*y + 127.5)).
    The u8 SATURATION implements the hardtanh clip; round-to-nearest was
    verified on HW. Host dequant: sp = (u8-127.5)/127.5 (max err 3.9e-3).
  - DVE scalar_tensor_tensor: z = (sp_u8 - 127.5) * (d/127.5)  [= sp*d].
  - ACT: th = tanh(z/2).  DVE stt: gini_u8 = (th*255)*th (round+sat).
    Host: gini = 1.5 - 0.5*u8/255.
  - Outputs staged [2(l-half), 128(l), N(rows)] u8; 4 KiB-per-partition
    stores every 4096 rows. Host transposes u8 slabs on dequant.
  - Blocks 1024/2048 rows: small first blocks cut the DMA ramp, small last
    blocks shorten the drain tail.
"""

import os
import sys
import types
from concurrent.futures import ThreadPoolExecutor

import numpy as np

for _p in (
    "/opt/trn_rl_repo",
    "/root/.axon_site",
    "/root/.axon_site/_ro/trn_rl_repo",
    "/root/.axon_site/_ro/pypackages",
):
    if os.path.isdir(_p) and _p not in sys.path:
        sys.path.append(_p)

B, T, F, L = 1024, 128, 784, 256
NCORES = 8
BS = B // NCORES          # batches per core
NROWS = BS * T            # 16384 rows per core
KT = 7                    # contraction tiles (784 = 6*128 + 16, padded)
KP = 17                   # used partitions in the last (remainder+bias) k-tile
CG = 2048                 # compute-group rows (elementwise op width)
CH = 512                  # matmul chunk rows (one psum bank)
DZW = 1024                # dz tile reps (d pattern is 128-periodic in rows)
BLOCKS = [512, 1024, 1536] + [2048] * 6 + [1024]


def _build_module():
    """Build + compile the single-core Bass/Tile module (SPMD across cores)."""
    import concourse.tile as tile
    from concourse import bacc, mybir

    f32, f16, u8 = mybir.dt.float32, mybir.dt.float16, mybir.dt.uint8
    Alu = mybir.AluOpType
    Act = mybir.ActivationFunctionType

    nc = bacc.Bacc(
        "TRN2",
        target_bir_lowering=False,
        debug=False,
        enable_asserts=False,
        num_devices=NCORES,
    )
    # wt/dz arrive pre-transposed from the host so their loads are single
    # linear 4 KiB-per-partition DMAs (a `rearrange` load costs ~900 tiny
    # 512 B descriptors).
    xt_d = nc.dram_tensor("xt", [KT, 128, NROWS], f16, kind="ExternalInput").ap()
    wt_d = nc.dram_tensor("wt", [128, KT * L], f16, kind="ExternalInput").ap()
    dz_d = nc.dram_tensor("dz", [128, 2 * DZW], f16, kind="ExternalInput").ap()
    sp_d = nc.dram_tensor("sp", [2, 128, NROWS], u8, kind="ExternalOutput").ap()
    gi_d = nc.dram_tensor("gini", [2, 128, NROWS], u8, kind="ExternalOutput").ap()

    # block start offsets
    starts = []
    n0 = 0
    for nb in BLOCKS:
        starts.append(n0)
        n0 += nb
    assert n0 == NROWS

    # compute groups: small final groups shorten the post-matmul drain
    CGS = [CG] * 7 + [1024, 1024]
    CG_OFF = []
    CG_END = []
    n0 = 0
    for w in CGS:
        CG_OFF.append(n0)
        n0 += w
        CG_END.append(n0)
    assert n0 == NROWS

    def block_of(r):
        for i in range(len(BLOCKS) - 1, -1, -1):
            if r >= starts[i]:
                return i
        raise AssertionError

    with tile.TileContext(nc) as tc:
        with (
            tc.tile_pool(name="consts", bufs=1) as consts,
            tc.tile_pool(name="xt", bufs=4) as xt_pool,
            tc.tile_pool(name="psum", bufs=8, space="PSUM") as psum_pool,
            tc.tile_pool(name="big", bufs=2) as big_pool,
            tc.tile_pool(name="tmp", bufs=2) as tmp_pool,
        ):
            # Persistent last-k-tile buffers: rows 17..127 stay zero so the
            # moving operand always spans 128 partitions; only the 17 real
            # rows are re-DMAed per block (triple-buffered). Memsets first:
            # no deps, and they gate the k6 DMAs of the first blocks.
            xk6s = []
            _ms_engines = [nc.gpsimd, nc.vector, nc.vector]
            for i in range(3):
                t6 = consts.tile([128, 2048], f16, tag=f"xk6_{i}")
                _ms_engines[i].memset(t6[:], 0.0)
                xk6s.append(t6)
            # wt first on the x-load (sync) queue: every matmul needs it;
            # dz later on the scalar queue (first use is ~15us in).
            wt_sb = consts.tile([128, KT, L], f16)
            nc.sync.dma_start(wt_sb[:].rearrange("p k l -> p (k l)"), wt_d[:])
            dz_sb = consts.tile([128, 2, DZW], f16)

            def emit_tail(off, w, sp_ts):
                """Elementwise tail + stores for the cg at rows [off, off+w)
                (runs one cg behind the matmul/ACT front so no engine queue
                ever blocks on a not-yet-ready dependency). Ops are emitted
                engine-major (all z, all tanh, all gini) so the chains of the
                two l-halves overlap across engines; the z of l-half 1 runs
                on GpSimd to take load off DVE."""
                TC = 512 if w <= 1024 else 1024
                pieces = []
                for nh in range(2):
                    for hf in range(w // TC):
                        sl = slice(hf * TC, (hf + 1) * TC)
                        z = tmp_pool.tile(
                            [128, TC], f16, tag=f"z{nh}{hf}", name="z"
                        )
                        nc.vector.scalar_tensor_tensor(
                            z[:],
                            sp_ts[nh][:, sl],
                            127.5,
                            dz_sb[:, nh, :TC],
                            Alu.subtract,
                            Alu.mult,
                        )
                        pieces.append((nh, sl, z))
                ths = []
                for nh, sl, z in pieces:
                    th = tmp_pool.tile(
                        [128, TC], f16, tag=f"th{nh}{sl.start // TC}", name="th"
                    )
                    nc.scalar.activation(th[:], z[:], Act.Tanh, scale=0.5)
                    ths.append((nh, sl, th))
                gi_ts = [
                    big_pool.tile([128, w], u8, tag=f"gi{h}", name=f"gi{h}")
                    for h in range(2)
                ]
                for nh, sl, th in ths:
                    nc.vector.scalar_tensor_tensor(
                        gi_ts[nh][:, sl], th[:], 255.0, th[:], Alu.mult, Alu.mult
                    )
                for nh in range(2):
                    nc.gpsimd.dma_start(
                        sp_d[nh, :, off : off + w], sp_ts[nh][:]
                    )
                    nc.gpsimd.dma_start(
                        gi_d[nh, :, off : off + w], gi_ts[nh][:]
                    )

            xts_by_block = {}
            pending_tail = None
            emitted_cg = 0
            for bi, nb in enumerate(BLOCKS):
                b0 = starts[bi]
                xts = []
                # For the first two blocks the per-DMA descriptor-generation
                # cost (~0.65us each) serializes the ramp on the sync queue:
                # spread issues across otherwise-idle engines.
                if bi < 2:
                    k_eng = [nc.sync, nc.sync, nc.scalar, nc.scalar,
                             nc.gpsimd, nc.gpsimd]
                else:
                    k_eng = [nc.sync] * 6
                for k in range(KT - 1):
                    xk = xt_pool.tile([128, nb], f16, tag=f"x{k}", bufs=5)
                    k_eng[k].dma_start(xk[:], xt_d[k, :, b0 : b0 + nb])
                    xts.append(xk)
                xk6 = xk6s[bi % 3]
                nc.sync.dma_start(xk6[0:KP, :nb], xt_d[KT - 1, 0:KP, b0 : b0 + nb])
                xts.append(xk6)
                xts_by_block[bi] = xts
                if bi == 1:
                    # dz load off the ramp's critical path
                    nc.scalar.dma_start(
                        dz_sb[:].rearrange("p n f -> p (n f)"), dz_d[:]
                    )

                while emitted_cg < len(CGS) and CG_END[emitted_cg] <= b0 + nb:
                    off, w = CG_OFF[emitted_cg], CGS[emitted_cg]
                    sp_ts = []
                    for nh in range(2):
                        sp_t = big_pool.tile([128, w], u8, tag=f"sp{nh}")
                        sp_ts.append(sp_t)
                        for ch in range(w // CH):
                            r = off + ch * CH
                            cb = block_of(r)
                            lo = r - starts[cb]
                            cxts = xts_by_block[cb]
                            ps = psum_pool.tile([128, CH], f32)
                            for k in range(KT):
                                nc.tensor.matmul(
                                    ps[:],
                                    wt_sb[:, k, nh * 128 : (nh + 1) * 128],
                                    cxts[k][:, lo : lo + CH],
                                    start=(k == 0),
                                    stop=(k == KT - 1),
                                )
                            nc.scalar.activation(
                                sp_t[:, ch * CH : (ch + 1) * CH],
                                ps[:],
                                Act.Copy,
                                bias=127.5,
                                scale=127.5,
                            )
                    if pending_tail is not None:
                        emit_tail(*pending_tail)
                    pending_tail = (off, w, sp_ts)
                    emitted_cg += 1
            if pending_tail is not None:
                emit_tail(*pending_tail)

    nc.compile()
    return nc


def _prep_core_x(x_flat_core):
    """[16384, 784] fp32 -> transposed fp16 [7, 128, 16384] (f on partitions).

    Row 16 of the last k-tile is the all-ones bias-fold row.
    """
    n = x_flat_core.shape[0]
    xsT16 = x_flat_core.T.astype(np.float16)  # [784, n], one strided pass
    xt = np.zeros((KT, 128, n), np.float16)
    xt[:6] = xsT16[:768].reshape(6, 128, n)
    xt[6, :16] = xsT16[768:784]
    xt[6, 16] = 1.0
    return xt


def _prep_wt(W, b):
    wt = np.zeros((KT, 128, L), np.float16)
    WT = W.T  # [784, 256]
    for k in range(6):
        wt[k] = WT[k * 128 : (k + 1) * 128]
    wt[6, :16] = WT[768:784]
    wt[6, 16] = b
    # device layout [128, KT*L]: partition = f-within-tile, free = (k, l)
    return np.ascontiguousarray(wt.transpose(1, 0, 2).reshape(128, KT * L))


_module_cache = {}


def _get_module():
    if "m" not in _module_cache:
        _module_cache["m"] = _build_module()
    return _module_cache["m"]


def _install_ntff_hook():
    """Register the axon NTFF profiling hook missing from this image's antenv."""
    try:
        import antenv.axon_hooks  # noqa: F401

        return
    except ImportError:
        pass
    try:
        from trn_agent_boot.trn_boot import _ntff_profile_via_ctypes

        hook = _ntff_profile_via_ctypes("/opt/axon/libaxon_pjrt.so")
    except Exception:
        hook = None
    mod = types.ModuleType("antenv.axon_hooks")
    mod.get_axon_ntff_profile_hook = lambda: hook
    mod.set_axon_ntff_profile_hook = lambda h: None
    sys.modules["antenv.axon_hooks"] = mod


def _unstage_core(args):
    """[2,128,NROWS] u8 pair -> (sp [NROWS,256] f32, gini [NROWS,256] f32)."""
    sp_u8, gi_u8 = args
    sp = (sp_u8.reshape(L, NROWS).T.astype(np.float32) - 127.5) * (1.0 / 127.5)
    gi = 1.5 - gi_u8.reshape(L, NROWS).T.astype(np.float32) * (0.5 / 255.0)
    return sp, gi


def _run(x, W, b, contribution, trace=False, tmpdir=None):
    from concourse import bass_utils

    nc = _get_module()

    x_flat = np.ascontiguousarray(x, dtype=np.float32).reshape(NCORES, NROWS, F)
    wt = _prep_wt(np.asarray(W, np.float32), np.asarray(b, np.float32))
    c = np.asarray(contribution, np.float32)
    d = c[:, :, 0] - c[:, :, 1]                      # [T, L]
    dT = (d.T * (1.0 / 127.5)).astype(np.float16)    # [L, T]
    # host layout [128, 2*DZW]: partition p holds [nh=0 reps | nh=1 reps]
    dz = np.ascontiguousarray(
        np.broadcast_to(
            dT.reshape(2, 128, 1, 128).transpose(1, 0, 2, 3),
            (128, 2, DZW // 128, 128),
        ).reshape(128, 2 * DZW)
    )

    with ThreadPoolExecutor(NCORES) as ex:
        xts = list(ex.map(_prep_core_x, [x_flat[i] for i in range(NCORES)]))

    if trace:
        _install_ntff_hook()
    in_maps = [{"xt": xts[i], "wt": wt, "dz": dz} for i in range(NCORES)]
    res = bass_utils.run_bass_kernel_spmd(
        nc, in_maps, core_ids=list(range(NCORES)), trace=trace, tmpdir=tmpdir
    )

    with ThreadPoolExecutor(NCORES) as ex:
        outs = list(
            ex.map(
                _unstage_core,
                [
                    (res.results[i]["sp"], res.results[i]["gini"])
                    for i in range(NCORES)
                ],
            )
        )
    sp = np.concatenate([o[0] for o in outs]).reshape(B, T, L)
    gini = np.concatenate([o[1] for o in outs]).reshape(B, T, L)
    out = (sp, gini)
    return (out, res) if trace else (out, None)


def kernel(x, W, b, contribution):
    out, _ = _run(x, W, b, contribution, trace=False)
    return out
